# revision 42
# baseline (speedup 1.0000x reference)
"""Trainium2 Bass kernel for DigitConvolutionalModel.

Model: x[B,784] -> reshape 28x28 -> 3x3 valid conv (weights conv_w) ->
[B,676] -> Linear(676,100)+relu -> Linear(100,10)+relu -> Linear(10,10).

The conv is linear, so it folds into the first Linear: W1f = C @ w1 where
C[784,676] is the conv unfold matrix. The whole model becomes a 3-layer MLP
784 -> 100 -> 10 -> 10 with relu between layers.

Sharding: pure data parallel, batch split across 8 cores (8192 rows each).

Precision: x is cast host-side to fp8 e3m4 (4 mantissa bits) — halves HBM
traffic vs bf16; weights stay bf16 (mixed-dtype matmul streams at the same
1 cycle/row). Measured end-to-end rel err 0.0142 vs the 2e-2 gate.

PE work per 512-batch supertile t is SEVEN 512-row matmuls:
  - 6 L1 main chunks (128 features each, features 0..767), accumulating
    into PSUM bank(t) rows 0-99 with start=False.
  - 1 "fused" matmul with block stationary S[126,126]:
        rows   0..99  = W2   -> out cols 100..109   (L2 of supertile t)
        rows 100..109 = W3   -> out cols 116..125   (L3 of supertile t-4)
        rows 110..125 = W1t  -> out cols   0..99    (L1 tail of t+4)
    moving operand fmov(t)[126, 512] (bf16, slot t of one persistent tile):
        rows   0..99  = h1(t)      (ACT relu of bank(t) rows 0..99)
        rows 100..109 = h2(t-4)    (ACT relu of an earlier fused output)
        rows 110..125 = xtail(t+4) (features 768..783, one slot-arranged
                                    DMA at kernel start)
    out = PSUM bank(t+4): rows 0..99 initialize the L1 accumulation for
    supertile t+4 (start=True; its chunks follow with start=False), rows
    100..109 -> relu -> h2(t), rows 116..125 + b3 -> y(t-4) -> store.
  Banks 0..3 are seeded by 4 standalone W1t tail matmuls at the start;
  supertiles 12..15 drain through 6 extra fused passes (scratch banks).

Biases live in the weight blob at the partition rows where the ACT engine
reads them: b1 rows 0-99, b2 rows 100-109, b3 rows 116-125. Engine APs
need a base partition in {0,32,64,96}, so the small h2/y ACTs run from
base 96: h2 as a 14-row op [96:110) whose garbage lanes 96..99 are
overwritten by the (later) h1 ACT of the same fmov slot — crucially it
stops BEFORE the xtail rows at 110+ — and y as a 30-row op [96:126)
into a private tile where only rows 116..125 are stored.
"""

import numpy as np
import ml_dtypes

import concourse.bacc as bacc
import concourse.tile as tile
from concourse.tile import add_dep_helper
from concourse import mybir
from concourse.bass_utils import run_bass_kernel_spmd

N_CORES = 8
B = 65536
BC = B // N_CORES  # 8192 rows per core
TN = 512           # batch columns per supertile
NT = BC // TN      # 16 supertiles per core
NKC = 6            # full 128-feature chunks (0..767)
KT = 16            # tail features (768..783)
NF = 784
H1 = 100
HO = 10
F32 = mybir.dt.float32
BF16 = mybir.dt.bfloat16
F8E3 = mybir.dt.float8e3
NP_BF16 = ml_dtypes.bfloat16
NP_F8E3 = ml_dtypes.float8_e3m4

# packed weight blob column layout (bf16 columns)
_C_W1M = 0                      # [128, 600]  w1m chunks
_C_FS = 600                     # [126, 126]  fused stationary S
_C_W1T = 726                    # [16, 100]   w1t (standalone, banks 0-3)
_C_B = 826                      # [126, 2]    b1/b3/b2 f32 byte-pairs by row
WBW = 828

NPAIR = NT // 2


def _build_nc():
    nc = bacc.Bacc(None, target_bir_lowering=False)

    xt_main = nc.dram_tensor(
        "xt_main", [NT, 128, NKC, TN], F8E3, kind="ExternalInput"
    )
    # tails arranged by fmov slot s (holding xtail(s+4); zeros for s>=12),
    # bf16 so they ride the bf16 fmov tile
    xt_tail = nc.dram_tensor("xt_tail", [KT, 22, TN], BF16, kind="ExternalInput")
    # tails 0..3 for the standalone bank-seed matmuls
    xt_tl03 = nc.dram_tensor("xt_tl03", [KT, 4, TN], BF16, kind="ExternalInput")
    wblob = nc.dram_tensor("wblob", [128, WBW], BF16, kind="ExternalInput")
    yt = nc.dram_tensor("yt", [HO, BC], F32, kind="ExternalOutput")

    relu = mybir.ActivationFunctionType.Relu
    ident = mybir.ActivationFunctionType.Identity

    with tile.TileContext(nc) as tc:
        with (
            tc.tile_pool(name="const", bufs=1) as cpool,
            tc.tile_pool(name="xm", bufs=6) as xpool,
            tc.tile_pool(name="fm", bufs=6) as fpool,
            tc.tile_pool(name="ot", bufs=4) as opool,
            tc.tile_pool(name="psA", bufs=6, space="PSUM") as psA,
        ):
            # weights first on the sync queue (small now: 207KB)
            wb_s = cpool.tile([128, WBW], BF16, tag="wb")
            nc.sync.dma_start(wb_s[:], wblob[:])
            xtl03 = cpool.tile([KT, 4, TN], BF16, tag="xtl03")
            nc.gpsimd.dma_start(xtl03[:], xt_tl03[:])

            # fused-moving tiles are pooled PER PAIR (dep tracking
            # intersects partition ranges coarsely — one persistent tile
            # would make every fused matmul wait on the latest h1/h2
            # write to ANY slot, serializing PE behind ACT each pair).
            # ptile q covers fmov slots 2q / 2q+1; its writers (h1-ACT of
            # pair q, h2-DVE from pair q-2, tail DMA) all complete at
            # least a pair before fused reads it.
            ptiles: dict[int, object] = {}

            def alloc_ptile(q):
                pt = fpool.tile([126, 2, TN], BF16, tag="fm",
                                name=f"ptile{q}")
                ptiles[q] = pt
                # tails for fmov slots 2q/2q+1 (zeros for slots >= 12)
                nc.gpsimd.dma_start(pt[110:126, :, :],
                                    xt_tail[:, 2 * q:2 * q + 2, :])
                if q <= 2:
                    # no h2(-6..-1) exist for fused(0..5)
                    nc.vector.memset(pt[96:110, :, :], 0.0)
                if q >= 8:
                    # drain slots 16..19: h1 region is never produced
                    nc.vector.memset(pt[0:H1, :, :], 0.0)
                return pt

            def fmov(s):
                return ptiles[s // 2][:, s % 2, :]

            fs_ap = wb_s[0:126, _C_FS:_C_FS + 126]
            w1t_ap = wb_s[0:KT, _C_W1T:_C_W1T + H1]
            b1_ap = wb_s[0:H1, _C_B:_C_B + 2].bitcast(F32)
            # engine APs need a base partition in {0,32,64,96}: the small
            # h2/y ACTs run from base 96 (lanes are parallel, width free);
            # rows 96..99 are garbage lanes, overwritten (fmov h1) or never
            # stored (ot). One bias column serves all: b1 rows 0-99,
            # b2 rows 100-109, b3 rows 116-125.
            bq2_ap = wb_s[96:110, _C_B:_C_B + 2].bitcast(F32)
            bq_ap = wb_s[96:126, _C_B:_C_B + 2].bitcast(F32)

            prev_mm = [None]

            def mm(out_ap, lhsT_ap, rhs_ap, start, stop, ldw=True):
                m = nc.tensor.matmul(out_ap, lhsT_ap, rhs_ap,
                                     start=start, stop=stop,
                                     skip_group_check=True)
                if not ldw:
                    m.ins.ldweights = False
                if prev_mm[0] is not None:
                    add_dep_helper(m.ins, prev_mm[0], sync=False,
                                   reason="pe program order")
                prev_mm[0] = m.ins
                return m

            # Short warmup: covers engine bring-up until the first real
            # operands land (~1.5us); PE pstate then ramps on real work.
            wsc = cpool.tile([128, TN], BF16, tag="wsc")
            wp0 = psA.tile([126, TN], F32, tag="pa")
            wp1 = psA.tile([126, TN], F32, tag="pa")
            wfirst = nc.tensor.matmul(wp0[:], wsc[:, 0:126], wsc[:],
                                      start=True, stop=True)
            for i in range(1, 3):
                w_mm = nc.tensor.matmul((wp1 if i % 2 else wp0)[:],
                                        wsc[:, 0:126], wsc[:],
                                        start=True, stop=True)
                w_mm.ins.ldweights = False
                add_dep_helper(w_mm.ins, wfirst.ins, sync=False,
                               reason="warmup weight reuse")
            # WAR on purpose: warmup multiplies garbage; the memset only
            # exists to satisfy tile allocation and runs afterwards.
            nc.vector.memset(wsc[:], 0.0)

            banks: dict[int, object] = {}
            ots: dict[int, object] = {}

            alloc_ptile(0)
            alloc_ptile(1)

            # Seed banks 0..3 with their L1 tail contribution.
            for t in range(4):
                banks[t] = psA.tile([126, TN], F32, tag="pa", name=f"bank{t}")
                mm(banks[t][0:H1, :], w1t_ap, xtl03[:, t, :],
                   start=True, stop=False, ldw=(t == 0))

            def emit_fused(s, ldw):
                """fused(s): L2(s) + L3(s-4) + L1-tail(s+4) -> bank(s+4).

                L3 lags by 4 so h2-ACT(s) -> fmov(s+4) lands ~1.5 pairs
                before h1-ACT(s+4) overwrites that slot's garbage rows
                96..99 — emission order alone gives the right WAW order.
                """
                bk = psA.tile([126, TN], F32, tag="pa", name=f"bank{s+4}")
                banks[s + 4] = bk
                mm(bk[:], fs_ap, fmov(s),
                   start=True, stop=(s + 4 > 15), ldw=ldw)
                if s <= 15:
                    # h2(s) -> fmov(s+6) rows 100..109 (14-row base-96 op;
                    # rows 96..99 garbage, overwritten by h1-ACT(s+6);
                    # stops BEFORE the xtail rows at 110+). On DVE: ops
                    # cost ~free-size regardless of rows, and the scalar
                    # queue is busy with the h1 relus + y adds. The read
                    # of rows 96..99 RAW-depends on bank(s+4)'s own
                    # chunks (same pair) — hence the SIX-supertile h2 lag,
                    # which leaves ~3 pairs of slack before fused(s+6).
                    nc.vector.scalar_tensor_tensor(
                        ptiles[(s + 6) // 2][96:110, s % 2, :],
                        bk[96:110, :], bq2_ap, wsc[96:110, :],
                        op0=mybir.AluOpType.add, op1=mybir.AluOpType.max)
                if s >= 6:
                    ot = opool.tile([126, TN], F32, tag="ot", name=f"ot{s-6}")
                    nc.scalar.activation(ot[96:126, :], bk[96:126, :],
                                         ident, bias=bq_ap)
                    # stores must not sit in the sync (loads) in-order
                    # queue: a store waiting on the L3 chain would gate
                    # later x-tile loads. gpsimd is idle. The drain
                    # stores go on sync, idle and lower-latency by then.
                    eng = nc.sync if s >= 16 else nc.gpsimd
                    eng.dma_start(
                        yt[:, (s - 6) * TN:(s - 5) * TN], ot[116:126, :]
                    )
                    ots[s - 6] = ot

            for p in range(NPAIR):
                t0, t1 = 2 * p, 2 * p + 1
                alloc_ptile(p + 2)
                fm = ptiles[p]
                # fused passes for the pair-before-last: every dependency
                # (h1-ACT of pair p-2, h2 writes from pair p-1) is at
                # least a full pair old, so the PE never waits here, and
                # the y-ACTs land in the scalar queue's idle early-pair
                # window, ahead of the late-pair h1 relus.
                if p >= 2:
                    emit_fused(2 * p - 4, ldw=True)
                    emit_fused(2 * p - 3, ldw=False)
                xm0 = xpool.tile([128, NKC, TN], F8E3, tag="xm")
                xm1 = xpool.tile([128, NKC, TN], F8E3, tag="xm")
                if p == 0:
                    # split first supertile so chunk 0 can start earlier
                    nc.sync.dma_start(xm0[:, 0:3, :], xt_main[t0, :, 0:3, :])
                    nc.sync.dma_start(xm0[:, 3:6, :], xt_main[t0, :, 3:6, :])
                else:
                    nc.sync.dma_start(xm0[:], xt_main[t0])
                nc.sync.dma_start(xm1[:], xt_main[t1])

                if p == NPAIR - 1:
                    # last pair: all of supertile 15 first so its h1/h2
                    # chain completes during supertile 14's chunks,
                    # shortening the drain.
                    for k in range(NKC):
                        mm(banks[t1][0:H1, :],
                           wb_s[:, k * H1:(k + 1) * H1],
                           xm1[:, k, :], start=False, stop=(k == NKC - 1))
                    nc.scalar.activation(fm[0:H1, 1, :],
                                         banks[t1][0:H1, :],
                                         relu, bias=b1_ap)
                    for k in range(NKC):
                        mm(banks[t0][0:H1, :],
                           wb_s[:, k * H1:(k + 1) * H1],
                           xm0[:, k, :], start=False, stop=(k == NKC - 1))
                    nc.scalar.activation(fm[0:H1, 0, :],
                                         banks[t0][0:H1, :],
                                         relu, bias=b1_ap)
                else:
                    for k in range(NKC):
                        mm(banks[t0][0:H1, :],
                           wb_s[:, k * H1:(k + 1) * H1],
                           xm0[:, k, :], start=False, stop=(k == NKC - 1))
                        mm(banks[t1][0:H1, :],
                           wb_s[:, k * H1:(k + 1) * H1],
                           xm1[:, k, :], start=False, stop=(k == NKC - 1),
                           ldw=False)
                    nc.scalar.activation(fm[0:H1, 0, :],
                                         banks[t0][0:H1, :],
                                         relu, bias=b1_ap)
                    nc.scalar.activation(fm[0:H1, 1, :],
                                         banks[t1][0:H1, :],
                                         relu, bias=b1_ap)
                del banks[t0], banks[t1]

            # drain: fused(12..21) produce y(6..15)
            alloc_ptile(10)
            for s in range(12, 22):
                emit_fused(s, ldw=(s == 12))

    nc.compile()
    return nc


def _fold_conv_into_w1(conv_w: np.ndarray, w1: np.ndarray) -> np.ndarray:
    """W1f[784,100] such that x @ W1f == conv(x).reshape(B,676) @ w1."""
    c = np.zeros((NF, 26 * 26), dtype=np.float64)
    for di in range(3):
        for dj in range(3):
            ii, jj = np.meshgrid(np.arange(26), np.arange(26), indexing="ij")
            src = (ii + di) * 28 + (jj + dj)
            dst = ii * 26 + jj
            c[src.ravel(), dst.ravel()] += np.float64(conv_w[di, dj])
    return (c @ w1.astype(np.float64)).astype(np.float32)


def _prep_in_maps(x, conv_w, w1, b1, w2, b2, w3, b3):
    x = np.asarray(x, dtype=np.float32)
    conv_w = np.asarray(conv_w, dtype=np.float32)
    w1 = np.asarray(w1, dtype=np.float32)
    b1 = np.asarray(b1, dtype=np.float32)
    w2 = np.asarray(w2, dtype=np.float32)
    b2 = np.asarray(b2, dtype=np.float32)
    w3 = np.asarray(w3, dtype=np.float32)
    b3 = np.asarray(b3, dtype=np.float32)

    w1f = _fold_conv_into_w1(conv_w, w1)  # [784, 100]
    # main chunks: feature f = k*128 + p -> [128, 600]
    w1m = np.ascontiguousarray(
        w1f[: 128 * NKC].reshape(NKC, 128, H1).transpose(1, 0, 2)
    ).astype(NP_BF16).reshape(128, NKC * H1)
    w1t = w1f[128 * NKC:].astype(NP_BF16)  # [16, 100]

    blob = np.zeros((128, WBW), np.uint16)
    blob[:, _C_W1M:_C_W1M + NKC * H1] = w1m.view(np.uint16)
    # fused stationary S[126,126]
    s_blk = np.zeros((126, 126), np.float32)
    s_blk[0:H1, 100:110] = w2
    s_blk[100:110, 116:126] = w3
    s_blk[110:126, 0:H1] = w1t.astype(np.float32)
    blob[0:126, _C_FS:_C_FS + 126] = s_blk.astype(NP_BF16).view(np.uint16)
    blob[0:KT, _C_W1T:_C_W1T + H1] = w1t.view(np.uint16)
    bias_rows = np.zeros((126, 1), np.float32)
    bias_rows[0:H1, 0] = b1
    bias_rows[100:110, 0] = b2
    bias_rows[116:126, 0] = b3
    blob[0:126, _C_B:_C_B + 2] = bias_rows.view(np.uint16)
    shared = {"wblob": blob.view(NP_BF16)}

    xb = x.astype(NP_F8E3)  # cast once, full batch
    in_maps = []
    for core in range(N_CORES):
        xc = xb[core * BC:(core + 1) * BC]  # [8192, 784] f8e3
        xct = xc.reshape(NT, TN, NF).transpose(0, 2, 1)  # [NT, NF, TN]
        xt_main = np.ascontiguousarray(
            xct[:, : 128 * NKC].reshape(NT, NKC, 128, TN).transpose(0, 2, 1, 3)
        )  # [NT, 128, NKC, TN]
        tails = xct[:, 128 * NKC:].astype(NP_BF16)  # [NT, KT, TN]
        # fmov slot s holds xtail(s+4); slots 12..21 stay zero
        xt_tail = np.zeros((KT, 22, TN), NP_BF16)
        xt_tail[:, 0:12, :] = tails[4:16].transpose(1, 0, 2)
        xt_tl03 = np.ascontiguousarray(tails[0:4].transpose(1, 0, 2))
        in_maps.append({"xt_main": xt_main, "xt_tail": xt_tail,
                        "xt_tl03": xt_tl03, **shared})
    return in_maps


_NC = None


def _get_nc():
    global _NC
    if _NC is None:
        _NC = _build_nc()
    return _NC


def kernel(x, conv_w, w1, b1, w2, b2, w3, b3):
    in_maps = _prep_in_maps(x, conv_w, w1, b1, w2, b2, w3, b3)
    nc = _get_nc()
    res = run_bass_kernel_spmd(nc, in_maps, core_ids=list(range(N_CORES)))
    out = np.empty((B, HO), dtype=np.float32)
    for i in range(N_CORES):
        out[i * BC:(i + 1) * BC] = res.results[i]["yt"].T
    return out


if __name__ == "__main__":
    rng = np.random.default_rng(0)
    inputs = {
        "x": rng.standard_normal((B, NF), dtype=np.float32),
        "conv_w": np.ones((3, 3), dtype=np.float32),
        "w1": (rng.standard_normal((676, H1)) * 0.04).astype(np.float32),
        "b1": np.zeros(H1, dtype=np.float32),
        "w2": (rng.standard_normal((H1, HO)) * 0.1).astype(np.float32),
        "b2": np.zeros(HO, dtype=np.float32),
        "w3": (rng.standard_normal((HO, HO)) * 0.3).astype(np.float32),
        "b3": np.zeros(HO, dtype=np.float32),
    }
    out = kernel(**inputs)
    print(out.shape, out.dtype)


# revision 44
# speedup vs baseline: 1.1046x; 1.1046x over previous
"""Trainium2 Bass kernel for DigitConvolutionalModel.

Model: x[B,784] -> reshape 28x28 -> 3x3 valid conv (weights conv_w) ->
[B,676] -> Linear(676,100)+relu -> Linear(100,10)+relu -> Linear(10,10).

The conv is linear, so it folds into the first Linear: W1f = C @ w1 where
C[784,676] is the conv unfold matrix. The whole model becomes a 3-layer MLP
784 -> 100 -> 10 -> 10 with relu between layers.

Sharding: pure data parallel, batch split across 8 cores (8192 rows each).

Precision: x is cast host-side to fp8 e3m4 (4 mantissa bits) — halves HBM
traffic vs bf16; weights stay bf16 (mixed-dtype matmul streams at the same
1 cycle/row). Measured end-to-end rel err 0.0142 vs the 2e-2 gate.

PE work per 512-batch supertile t is SEVEN 512-row matmuls:
  - 6 L1 main chunks (128 features each, features 0..767), accumulating
    into PSUM bank(t) rows 0-99 with start=False.
  - 1 "fused" matmul with block stationary S[126,126]:
        rows   0..99  = W2   -> out cols 100..109   (L2 of supertile t)
        rows 100..109 = W3   -> out cols 116..125   (L3 of supertile t-4)
        rows 110..125 = W1t  -> out cols   0..99    (L1 tail of t+4)
    moving operand fmov(t)[126, 512] (bf16, slot t of one persistent tile):
        rows   0..99  = h1(t)      (ACT relu of bank(t) rows 0..99)
        rows 100..109 = h2(t-4)    (ACT relu of an earlier fused output)
        rows 110..125 = xtail(t+4) (features 768..783, one slot-arranged
                                    DMA at kernel start)
    out = PSUM bank(t+4): rows 0..99 initialize the L1 accumulation for
    supertile t+4 (start=True; its chunks follow with start=False), rows
    100..109 -> relu -> h2(t), rows 116..125 + b3 -> y(t-4) -> store.
  Banks 0..3 are seeded by 4 standalone W1t tail matmuls at the start;
  supertiles 12..15 drain through 6 extra fused passes (scratch banks).

Biases live in the weight blob at the partition rows where the ACT engine
reads them: b1 rows 0-99, b2 rows 100-109, b3 rows 116-125. Engine APs
need a base partition in {0,32,64,96}, so the small h2/y ACTs run from
base 96: h2 as a 14-row op [96:110) whose garbage lanes 96..99 are
overwritten by the (later) h1 ACT of the same fmov slot — crucially it
stops BEFORE the xtail rows at 110+ — and y as a 30-row op [96:126)
into a private tile where only rows 116..125 are stored.
"""

import numpy as np
import ml_dtypes

import concourse.bacc as bacc
import concourse.tile as tile
from concourse.tile import add_dep_helper
from concourse import mybir
from concourse.bass_utils import run_bass_kernel_spmd

N_CORES = 8
B = 65536
BC = B // N_CORES  # 8192 rows per core
TN = 512           # batch columns per supertile
NT = BC // TN      # 16 supertiles per core
NKC = 6            # full 128-feature chunks (0..767)
KT = 16            # tail features (768..783)
NF = 784
H1 = 100
HO = 10
F32 = mybir.dt.float32
BF16 = mybir.dt.bfloat16
F8E3 = mybir.dt.float8e3
NP_BF16 = ml_dtypes.bfloat16
NP_F8E3 = ml_dtypes.float8_e3m4

# packed weight blob column layout (bf16 columns)
_C_W1M = 0                      # [128, 600]  w1m chunks
_C_FS = 600                     # [126, 126]  fused stationary S
_C_W1T = 726                    # [16, 100]   w1t (standalone, banks 0-3)
_C_B = 826                      # [126, 2]    b1/b3/b2 f32 byte-pairs by row
WBW = 828

NPAIR = NT // 2


def _build_nc():
    nc = bacc.Bacc(None, target_bir_lowering=False)

    xt_main = nc.dram_tensor(
        "xt_main", [NT, 128, NKC, TN], F8E3, kind="ExternalInput"
    )
    # tails arranged by fmov slot s (holding xtail(s+4); zeros for s>=12),
    # bf16 so they ride the bf16 fmov tile
    xt_tail = nc.dram_tensor("xt_tail", [KT, 22, TN], BF16, kind="ExternalInput")
    # tails 0..3 for the standalone bank-seed matmuls
    xt_tl03 = nc.dram_tensor("xt_tl03", [KT, 4, TN], BF16, kind="ExternalInput")
    wblob = nc.dram_tensor("wblob", [128, WBW], BF16, kind="ExternalInput")
    yt = nc.dram_tensor("yt", [HO, BC], F32, kind="ExternalOutput")

    relu = mybir.ActivationFunctionType.Relu
    ident = mybir.ActivationFunctionType.Identity

    with tile.TileContext(nc) as tc:
        with (
            tc.tile_pool(name="const", bufs=1) as cpool,
            tc.tile_pool(name="xm", bufs=6) as xpool,
            tc.tile_pool(name="fm", bufs=6) as fpool,
            tc.tile_pool(name="ot", bufs=4) as opool,
            tc.tile_pool(name="psA", bufs=6, space="PSUM") as psA,
        ):
            # weights first on the sync queue (small now: 207KB)
            wb_s = cpool.tile([128, WBW], BF16, tag="wb")
            nc.sync.dma_start(wb_s[:], wblob[:])
            xtl03 = cpool.tile([KT, 4, TN], BF16, tag="xtl03")
            nc.gpsimd.dma_start(xtl03[:], xt_tl03[:])

            # fused-moving tiles are pooled PER PAIR (dep tracking
            # intersects partition ranges coarsely — one persistent tile
            # would make every fused matmul wait on the latest h1/h2
            # write to ANY slot, serializing PE behind ACT each pair).
            # ptile q covers fmov slots 2q / 2q+1; its writers (h1-ACT of
            # pair q, h2-DVE from pair q-2, tail DMA) all complete at
            # least a pair before fused reads it.
            ptiles: dict[int, object] = {}

            def alloc_ptile(q):
                pt = fpool.tile([126, 2, TN], BF16, tag="fm",
                                name=f"ptile{q}")
                ptiles[q] = pt
                # tails for fmov slots 2q/2q+1 (zeros for slots >= 12)
                nc.gpsimd.dma_start(pt[110:126, :, :],
                                    xt_tail[:, 2 * q:2 * q + 2, :])
                if q <= 2:
                    # no h2(-6..-1) exist for fused(0..5)
                    nc.vector.memset(pt[96:110, :, :], 0.0)
                if q >= 8:
                    # drain slots 16..19: h1 region is never produced
                    nc.vector.memset(pt[0:H1, :, :], 0.0)
                return pt

            def fmov(s):
                return ptiles[s // 2][:, s % 2, :]

            fs_ap = wb_s[0:126, _C_FS:_C_FS + 126]
            w1t_ap = wb_s[0:KT, _C_W1T:_C_W1T + H1]
            b1_ap = wb_s[0:H1, _C_B:_C_B + 2].bitcast(F32)
            # engine APs need a base partition in {0,32,64,96}: the small
            # h2/y ACTs run from base 96 (lanes are parallel, width free);
            # rows 96..99 are garbage lanes, overwritten (fmov h1) or never
            # stored (ot). One bias column serves all: b1 rows 0-99,
            # b2 rows 100-109, b3 rows 116-125.
            bq2_ap = wb_s[96:110, _C_B:_C_B + 2].bitcast(F32)
            bq_ap = wb_s[96:126, _C_B:_C_B + 2].bitcast(F32)

            prev_mm = [None]

            def mm(out_ap, lhsT_ap, rhs_ap, start, stop, ldw=True):
                m = nc.tensor.matmul(out_ap, lhsT_ap, rhs_ap,
                                     start=start, stop=stop,
                                     skip_group_check=True)
                if not ldw:
                    m.ins.ldweights = False
                if prev_mm[0] is not None:
                    add_dep_helper(m.ins, prev_mm[0], sync=False,
                                   reason="pe program order")
                prev_mm[0] = m.ins
                return m

            # Short warmup: covers engine bring-up until the first real
            # operands land (~1.5us); PE pstate then ramps on real work.
            wsc = cpool.tile([128, TN], BF16, tag="wsc")
            wp0 = psA.tile([126, TN], F32, tag="pa")
            wp1 = psA.tile([126, TN], F32, tag="pa")
            wfirst = nc.tensor.matmul(wp0[:], wsc[:, 0:126], wsc[:],
                                      start=True, stop=True)
            for i in range(1, 3):
                w_mm = nc.tensor.matmul((wp1 if i % 2 else wp0)[:],
                                        wsc[:, 0:126], wsc[:],
                                        start=True, stop=True)
                w_mm.ins.ldweights = False
                add_dep_helper(w_mm.ins, wfirst.ins, sync=False,
                               reason="warmup weight reuse")
            # WAR on purpose: warmup multiplies garbage; the memset only
            # exists to satisfy tile allocation and runs afterwards.
            nc.vector.memset(wsc[:], 0.0)

            banks: dict[int, object] = {}
            ots: dict[int, object] = {}

            alloc_ptile(0)
            alloc_ptile(1)

            # Seed banks 0..3 with their L1 tail contribution.
            for t in range(4):
                banks[t] = psA.tile([126, TN], F32, tag="pa", name=f"bank{t}")
                mm(banks[t][0:H1, :], w1t_ap, xtl03[:, t, :],
                   start=True, stop=False, ldw=(t == 0))

            def emit_fused(s, ldw):
                """fused(s): L2(s) + L3(s-6) + L1-tail(s+4) -> bank(s+4)."""
                bk = psA.tile([126, TN], F32, tag="pa", name=f"bank{s+4}")
                banks[s + 4] = bk
                mm(bk[:], fs_ap, fmov(s),
                   start=True, stop=(s + 4 > 15), ldw=ldw)

            def emit_post(s):
                """h2/y reads of bank(s+4). Emitted AFTER the pair's
                chunks: their base-96 APs touch rows 96..99 of the bank,
                and emitting them first would WAR-block the chunk matmuls
                behind two ~700ns engine ops every pair."""
                bk = banks[s + 4]
                if s <= 15:
                    # h2(s) -> fmov(s+6) rows 100..109 (14-row base-96 op;
                    # rows 96..99 garbage, overwritten by h1-ACT(s+6);
                    # stops BEFORE the xtail rows at 110+). On DVE: ops
                    # cost ~free-size regardless of rows, and the scalar
                    # queue is busy with the h1 relus + y adds. The read
                    # of rows 96..99 RAW-depends on bank(s+4)'s own
                    # chunks (same pair) — hence the SIX-supertile h2 lag,
                    # which leaves ~3 pairs of slack before fused(s+6).
                    nc.vector.scalar_tensor_tensor(
                        ptiles[(s + 6) // 2][96:110, s % 2, :],
                        bk[96:110, :], bq2_ap, wsc[96:110, :],
                        op0=mybir.AluOpType.add, op1=mybir.AluOpType.max)
                if s >= 6:
                    ot = opool.tile([126, TN], F32, tag="ot", name=f"ot{s-6}")
                    nc.scalar.activation(ot[96:126, :], bk[96:126, :],
                                         ident, bias=bq_ap)
                    # stores must not sit in the sync (loads) in-order
                    # queue: a store waiting on the L3 chain would gate
                    # later x-tile loads. gpsimd is idle. The drain
                    # stores go on sync, idle and lower-latency by then.
                    eng = nc.sync if s >= 16 else nc.gpsimd
                    eng.dma_start(
                        yt[:, (s - 6) * TN:(s - 5) * TN], ot[116:126, :]
                    )
                    ots[s - 6] = ot

            for p in range(NPAIR):
                t0, t1 = 2 * p, 2 * p + 1
                alloc_ptile(p + 2)
                fm = ptiles[p]
                # fused passes for the pair-before-last: every dependency
                # (h1-ACT of pair p-2, h2 writes from pair p-1) is at
                # least a full pair old, so the PE never waits here, and
                # the y-ACTs land in the scalar queue's idle early-pair
                # window, ahead of the late-pair h1 relus.
                if p >= 2:
                    emit_fused(2 * p - 4, ldw=True)
                    emit_fused(2 * p - 3, ldw=False)
                xm0 = xpool.tile([128, NKC, TN], F8E3, tag="xm")
                xm1 = xpool.tile([128, NKC, TN], F8E3, tag="xm")
                if p == 0:
                    # split first supertile so chunk 0 can start earlier
                    nc.sync.dma_start(xm0[:, 0:3, :], xt_main[t0, :, 0:3, :])
                    nc.sync.dma_start(xm0[:, 3:6, :], xt_main[t0, :, 3:6, :])
                else:
                    nc.sync.dma_start(xm0[:], xt_main[t0])
                nc.sync.dma_start(xm1[:], xt_main[t1])

                if p == NPAIR - 1:
                    # last pair: all of supertile 15 first so its h1/h2
                    # chain completes during supertile 14's chunks,
                    # shortening the drain.
                    for k in range(NKC):
                        mm(banks[t1][0:H1, :],
                           wb_s[:, k * H1:(k + 1) * H1],
                           xm1[:, k, :], start=False, stop=(k == NKC - 1))
                    nc.scalar.activation(fm[0:H1, 1, :],
                                         banks[t1][0:H1, :],
                                         relu, bias=b1_ap)
                    for k in range(NKC):
                        mm(banks[t0][0:H1, :],
                           wb_s[:, k * H1:(k + 1) * H1],
                           xm0[:, k, :], start=False, stop=(k == NKC - 1))
                    nc.scalar.activation(fm[0:H1, 0, :],
                                         banks[t0][0:H1, :],
                                         relu, bias=b1_ap)
                else:
                    for k in range(NKC):
                        mm(banks[t0][0:H1, :],
                           wb_s[:, k * H1:(k + 1) * H1],
                           xm0[:, k, :], start=False, stop=(k == NKC - 1))
                        mm(banks[t1][0:H1, :],
                           wb_s[:, k * H1:(k + 1) * H1],
                           xm1[:, k, :], start=False, stop=(k == NKC - 1),
                           ldw=False)
                    nc.scalar.activation(fm[0:H1, 0, :],
                                         banks[t0][0:H1, :],
                                         relu, bias=b1_ap)
                    nc.scalar.activation(fm[0:H1, 1, :],
                                         banks[t1][0:H1, :],
                                         relu, bias=b1_ap)
                # deferred bank readers for this pair's fused outputs
                if p >= 2:
                    emit_post(2 * p - 4)
                    emit_post(2 * p - 3)
                del banks[t0], banks[t1]

            # drain: fused(12..21) produce y(6..15); scratch banks have
            # no chunks, so each post can follow its matmul directly
            alloc_ptile(10)
            for s in range(12, 22):
                emit_fused(s, ldw=(s == 12))
                emit_post(s)

    nc.compile()
    return nc


def _fold_conv_into_w1(conv_w: np.ndarray, w1: np.ndarray) -> np.ndarray:
    """W1f[784,100] such that x @ W1f == conv(x).reshape(B,676) @ w1."""
    c = np.zeros((NF, 26 * 26), dtype=np.float64)
    for di in range(3):
        for dj in range(3):
            ii, jj = np.meshgrid(np.arange(26), np.arange(26), indexing="ij")
            src = (ii + di) * 28 + (jj + dj)
            dst = ii * 26 + jj
            c[src.ravel(), dst.ravel()] += np.float64(conv_w[di, dj])
    return (c @ w1.astype(np.float64)).astype(np.float32)


def _prep_in_maps(x, conv_w, w1, b1, w2, b2, w3, b3):
    x = np.asarray(x, dtype=np.float32)
    conv_w = np.asarray(conv_w, dtype=np.float32)
    w1 = np.asarray(w1, dtype=np.float32)
    b1 = np.asarray(b1, dtype=np.float32)
    w2 = np.asarray(w2, dtype=np.float32)
    b2 = np.asarray(b2, dtype=np.float32)
    w3 = np.asarray(w3, dtype=np.float32)
    b3 = np.asarray(b3, dtype=np.float32)

    w1f = _fold_conv_into_w1(conv_w, w1)  # [784, 100]
    # main chunks: feature f = k*128 + p -> [128, 600]
    w1m = np.ascontiguousarray(
        w1f[: 128 * NKC].reshape(NKC, 128, H1).transpose(1, 0, 2)
    ).astype(NP_BF16).reshape(128, NKC * H1)
    w1t = w1f[128 * NKC:].astype(NP_BF16)  # [16, 100]

    blob = np.zeros((128, WBW), np.uint16)
    blob[:, _C_W1M:_C_W1M + NKC * H1] = w1m.view(np.uint16)
    # fused stationary S[126,126]
    s_blk = np.zeros((126, 126), np.float32)
    s_blk[0:H1, 100:110] = w2
    s_blk[100:110, 116:126] = w3
    s_blk[110:126, 0:H1] = w1t.astype(np.float32)
    blob[0:126, _C_FS:_C_FS + 126] = s_blk.astype(NP_BF16).view(np.uint16)
    blob[0:KT, _C_W1T:_C_W1T + H1] = w1t.view(np.uint16)
    bias_rows = np.zeros((126, 1), np.float32)
    bias_rows[0:H1, 0] = b1
    bias_rows[100:110, 0] = b2
    bias_rows[116:126, 0] = b3
    blob[0:126, _C_B:_C_B + 2] = bias_rows.view(np.uint16)
    shared = {"wblob": blob.view(NP_BF16)}

    xb = x.astype(NP_F8E3)  # cast once, full batch
    in_maps = []
    for core in range(N_CORES):
        xc = xb[core * BC:(core + 1) * BC]  # [8192, 784] f8e3
        xct = xc.reshape(NT, TN, NF).transpose(0, 2, 1)  # [NT, NF, TN]
        xt_main = np.ascontiguousarray(
            xct[:, : 128 * NKC].reshape(NT, NKC, 128, TN).transpose(0, 2, 1, 3)
        )  # [NT, 128, NKC, TN]
        tails = xct[:, 128 * NKC:].astype(NP_BF16)  # [NT, KT, TN]
        # fmov slot s holds xtail(s+4); slots 12..21 stay zero
        xt_tail = np.zeros((KT, 22, TN), NP_BF16)
        xt_tail[:, 0:12, :] = tails[4:16].transpose(1, 0, 2)
        xt_tl03 = np.ascontiguousarray(tails[0:4].transpose(1, 0, 2))
        in_maps.append({"xt_main": xt_main, "xt_tail": xt_tail,
                        "xt_tl03": xt_tl03, **shared})
    return in_maps


_NC = None


def _get_nc():
    global _NC
    if _NC is None:
        _NC = _build_nc()
    return _NC


def kernel(x, conv_w, w1, b1, w2, b2, w3, b3):
    in_maps = _prep_in_maps(x, conv_w, w1, b1, w2, b2, w3, b3)
    nc = _get_nc()
    res = run_bass_kernel_spmd(nc, in_maps, core_ids=list(range(N_CORES)))
    out = np.empty((B, HO), dtype=np.float32)
    for i in range(N_CORES):
        out[i * BC:(i + 1) * BC] = res.results[i]["yt"].T
    return out


if __name__ == "__main__":
    rng = np.random.default_rng(0)
    inputs = {
        "x": rng.standard_normal((B, NF), dtype=np.float32),
        "conv_w": np.ones((3, 3), dtype=np.float32),
        "w1": (rng.standard_normal((676, H1)) * 0.04).astype(np.float32),
        "b1": np.zeros(H1, dtype=np.float32),
        "w2": (rng.standard_normal((H1, HO)) * 0.1).astype(np.float32),
        "b2": np.zeros(HO, dtype=np.float32),
        "w3": (rng.standard_normal((HO, HO)) * 0.3).astype(np.float32),
        "b3": np.zeros(HO, dtype=np.float32),
    }
    out = kernel(**inputs)
    print(out.shape, out.dtype)
